# revision 60
# baseline (speedup 1.0000x reference)
"""Multi-head causal self-attention with RoPE on 8 Trainium2 NeuronCores.

Sharding: 12 heads over 8 cores. Core pairs (2p, 2p+1) share 3 heads:
  core 2p:   slot A = head 3p   (all 8 q-blocks), slot B = head 3p+1, q-blocks BSET_EVEN
  core 2p+1: slot A = head 3p+2 (all 8 q-blocks), slot B = head 3p+1, q-blocks BSET_ODD
B-set split (0,3,4,7)/(1,2,5,6) balances the Act-engine (exp) tail between programs.

Dataflow per core (all PE operands bf16):
  Q^T/K^T: [dh(2 heads)=128, S] (dh on partitions), K-stationary logits^T = K @ Q^T
  comes off the PE as [keys, queries].  exp on Act -> ex bf16 tiles [128k, q].
  AV is *ex-stationary*: out^T[q, d] accumulates per 128-query tile with moving
  operand [V | 1] (65 wide), so the softmax denominator lands in column 64 and the
  divide is a per-partition tensor_scalar.  V is projected directly into [token, dh]
  layout (x-tile stationary), no V transpose.  Attention output transposes back to
  [dh, q] on the PE, then O-projection partials [768, S] are staged fp16 and summed
  on the host.
RoPE pairs are (d, d+16) within each 32-partition quadrant (folded into a host-side
weight-row permutation) so the pair swap is one stream_shuffle rotate-16.
"""
import sys, os
sys.path.insert(0, "/opt/trn_rl_repo")
os.environ.setdefault("MYCRO_LOCAL_CACHE", "1")

import numpy as np

S, D, H, DH = 4096, 768, 12, 64
NCH, CH = 8, 512     # token chunks (projection phase) == query blocks
NQB, QB = 8, 512
NKT, KT = 32, 128    # key tiles
VPW = 130            # vp_all per-ktile width: [V_A(64) | 1 | V_B(64) | 1]
THETA = 10000.0
ROT16 = [(i + 16) % 32 for i in range(32)]

BSET_EVEN = (0, 3, 4, 7)
BSET_ODD = (1, 2, 5, 6)

# core -> (headA, headB)
CORE_HEADS = []
for p in range(4):
    CORE_HEADS.append((3 * p, 3 * p + 1))
    CORE_HEADS.append((3 * p + 2, 3 * p + 1))

# row r (0..63) inside a head slot -> original within-head dim.
# quadrant q = r//32, i = r%32: freq f = 16*q + (i%16); i<16 -> dim 2f, else 2f+1.
PERM64 = []
for r in range(64):
    q, i = r // 32, r % 32
    f = 16 * q + (i % 16)
    PERM64.append(2 * f if i < 16 else 2 * f + 1)
PERM64 = np.array(PERM64)

_PROGRAMS = {}


def _build_program(bset):
    import concourse.bass as bass
    import concourse.tile as tile
    from concourse import bacc, mybir
    from concourse.alu_op_type import AluOpType

    dt = mybir.dt
    F32, BF16, F16 = dt.float32, dt.bfloat16, dt.float16
    AF = mybir.ActivationFunctionType

    nc = bacc.Bacc("TRN2", target_bir_lowering=False, debug=False, num_devices=4)

    xt_d = nc.dram_tensor("xt", [D, S], BF16, kind="ExternalInput").ap()
    wqt_d = nc.dram_tensor("wqt", [D, 128], BF16, kind="ExternalInput").ap()
    wkt_d = nc.dram_tensor("wkt", [D, 128], BF16, kind="ExternalInput").ap()
    wvt_d = nc.dram_tensor("wvt", [D, 128], BF16, kind="ExternalInput").ap()
    wot_d = nc.dram_tensor("wot", [128, D], BF16, kind="ExternalInput").ap()
    cosf_d = nc.dram_tensor("cosf", [128, S], F32, kind="ExternalInput").ap()
    sins_d = nc.dram_tensor("sins", [128, S], F32, kind="ExternalInput").ap()
    tri_d = nc.dram_tensor("tri", [128, 128], BF16, kind="ExternalInput").ap()
    eye_d = nc.dram_tensor("eye", [128, 128], BF16, kind="ExternalInput").ap()
    opart_d = nc.dram_tensor("opart", [D, S], F16, kind="ExternalOutput").ap()
    DEBUG = bool(os.environ.get("MHA_DEBUG"))
    if DEBUG:
        dbg_kt = nc.dram_tensor("dbg_kt", [128, S], F32, kind="ExternalOutput").ap()
        dbg_qt = nc.dram_tensor("dbg_qt", [128, S], F32, kind="ExternalOutput").ap()
        dbg_vp = nc.dram_tensor("dbg_vp", [128, NKT * VPW], F32, kind="ExternalOutput").ap()
        dbg_at = nc.dram_tensor("dbg_at", [128, S], F32, kind="ExternalOutput").ap()

    sched = [(qb, [0] + ([1] if qb in bset else [])) for qb in range(NQB)]

    with tile.TileContext(nc) as tc:
        with (
            tc.tile_pool(name="const", bufs=1) as cp,
            tc.tile_pool(name="xc", bufs=2) as xcp,
            tc.tile_pool(name="rt", bufs=3) as rtp,
            tc.tile_pool(name="ex", bufs=6) as exp_pool,
            tc.tile_pool(name="dv", bufs=3) as dvp,
            tc.tile_pool(name="osb", bufs=3) as osb,
            tc.tile_pool(name="psL", bufs=2, space="PSUM") as psL,
            tc.tile_pool(name="psJ", bufs=2, space="PSUM") as psJ,
            tc.tile_pool(name="psAV", bufs=2, space="PSUM") as psAV,
        ):
            kt_rot = cp.tile([128, S], BF16, tag="ktrot")
            qt_rot = cp.tile([128, S], BF16, tag="qtrot")
            vp_all = cp.tile([128, NKT * VPW], BF16, tag="vpall")
            at_all = cp.tile([128, S], BF16, tag="atall")
            tri = cp.tile([128, 128], BF16, tag="tri")
            eye = cp.tile([128, 128], BF16, tag="eye")
            wo_all = cp.tile([128, D], BF16, tag="wo")
            wq_all = cp.tile([128, 6 * 128], BF16, tag="wqa")
            wk_all = cp.tile([128, 6 * 128], BF16, tag="wka")
            wv_all = cp.tile([128, 6 * 128], BF16, tag="wva")
            wq_t = [wq_all[:, i * 128:(i + 1) * 128] for i in range(6)]
            wk_t = [wk_all[:, i * 128:(i + 1) * 128] for i in range(6)]
            wv_t = [wv_all[:, i * 128:(i + 1) * 128] for i in range(6)]

            # ones columns of vp_all: cols {130t+64, 130t+129}
            nc.vector.memset(
                vp_all[:].rearrange("p (t x) -> p t x", x=VPW)[:, :, 64:VPW:65], 1.0)

            # warm-up matmuls: ramp the PE p-state inside the initial DMA
            # shadow so the first projections run at full clock
            wtile = cp.tile([128, 512], BF16, tag="wtile")
            nc.vector.memset(wtile[:], 0.0)
            for w in range(8):
                pw = psJ.tile([128, CH], F32, tag="pj", name=f"warm{w}")
                nc.tensor.matmul(pw[:], wtile[:, 0:128], wtile[:],
                                 start=True, stop=True)

            def dma_weights_back():
                nc.sync.dma_start(tri[:], tri_d[:])
                nc.sync.dma_start(eye[:], eye_d[:])
                nc.sync.dma_start(wo_all[:], wot_d[:])

            # -------- deferred work queues (proj pieces + O-proj tiles) ---
            # Emitted piecewise between attention groups so the PE queue never
            # holds a monolithic block that starves the Act engine.  Projection
            # pieces take priority: the next q-block stalls on them, while
            # O-projection tiles have a full q-block of slack.
            div_q = []
            proj_q = []
            oproj_q = []
            bucket = [0]

            def drain_ns(budget):
                while (div_q or proj_q or oproj_q) and budget > 0:
                    ent = (div_q or proj_q or oproj_q).pop(0)
                    ent[1]()
                    budget -= ent[0]
                return budget

            def drain_div():
                while div_q:
                    div_q.pop(0)[1]()

            def drain_proj_upto(c):
                while proj_q and proj_q[0][2] <= c:
                    proj_q.pop(0)[1]()

            def drain_all():
                while div_q or proj_q or oproj_q:
                    (div_q or proj_q or oproj_q).pop(0)[1]()

            def queue_oproj(c, fast=False):
                contr = 128 if c in bset else 64
                ot = osb.tile([128, 6 * CH], F16, tag="ot", name=f"ot{c}")
                halves = ((0, 512),)

                def mk(mt, h0, h1):
                    def emit():
                        po = psJ.tile([128, h1 - h0], F32, tag="pj",
                                      name=f"po{c}_{mt}_{h0}")
                        nc.tensor.matmul(
                            po[:], wo_all[0:contr, mt * 128:(mt + 1) * 128],
                            at_all[0:contr, c * CH + h0:c * CH + h1],
                            start=True, stop=True)
                        if fast and mt % 2 == 1:
                            nc.scalar.copy(
                                ot[:, mt * CH + h0:mt * CH + h1], po[:])
                        else:
                            nc.vector.tensor_copy(
                                ot[:, mt * CH + h0:mt * CH + h1], po[:])
                        if h1 == 512:
                            nc.sync.dma_start(
                                opart_d[mt * 128:(mt + 1) * 128,
                                        c * CH:(c + 1) * CH],
                                ot[:, mt * CH:(mt + 1) * CH])
                    return emit

                for (h0, h1) in halves:
                    for mt in range(6):
                        oproj_q.append((260, mk(mt, h0, h1)))

            # ---------------- attention -------------------------------
            def attention_qb(qb, slots, last):
                avs = {s: psAV.tile([128, 4 * 65], F32, tag="av",
                                    name=f"av{qb}_{s}")
                       for s in slots}
                nkt = 4 * (qb + 1)

                def emit_avs(exs_, ta_):
                    # ex-stationary AV: for each (slot, ktile, qtile) causal
                    # pair.  PSUM pending-zero tracking is BANK-granular, so
                    # each avs bank must hold a single accumulation group:
                    # start only on the first matmul into the tile (its
                    # bank-wide pending mark makes every region's first write
                    # an overwrite), stop only on the last.  h1 tiles may be
                    # packed left by `shift` (see the exp-width trim below).
                    for h_ in range(2):
                        t_ = ta_ + h_
                        m_ = t_ - 4 * qb
                        for s_ in slots:
                            ex_, shift_ = exs_[s_]
                            base_ = h_ * QB - (shift_ if h_ == 1 else 0)
                            for j_ in range(4):
                                if m_ > j_:
                                    continue  # strictly above diagonal
                                nc.tensor.matmul(
                                    avs[s_][:, j_ * 65:(j_ + 1) * 65],
                                    ex_[:, base_ + j_ * KT:
                                        base_ + (j_ + 1) * KT],
                                    vp_all[:, t_ * VPW + s_ * 65:
                                           t_ * VPW + s_ * 65 + 65],
                                    start=(t_ == 0 and j_ == 0),
                                    stop=(t_ == 4 * qb + 3))

                prev = None
                for g in range(nkt // 2):
                    ta = 2 * g
                    exs = {}
                    for s in slots:
                        lg = psL.tile([128, 2 * QB], F32, tag="lg",
                                      name=f"lg{qb}_{g}_{s}")
                        ex = exp_pool.tile([128, 2 * QB], BF16, tag="ex",
                                           name=f"ex{qb}_{g}_{s}")
                        start_col = 0
                        shift = 0
                        masks = []
                        for h in range(2):
                            t = ta + h
                            m = t - 4 * qb
                            off = 128 * m if m >= 0 else 0
                            if h == 0:
                                start_col = off
                                base = 0
                            else:
                                # pack h1's valid region against h0's end so
                                # the exp doesn't cover garbage columns
                                shift = off
                                base = QB - shift
                            nc.tensor.matmul(
                                lg[:, base + off:base + QB],
                                kt_rot[s * 64:(s + 1) * 64, t * KT:(t + 1) * KT],
                                qt_rot[s * 64:(s + 1) * 64,
                                       qb * QB + off:(qb + 1) * QB],
                                start=True, stop=True)
                            if m >= 0:
                                masks.append(base + off)
                        end_col = 2 * QB - shift
                        nc.scalar.activation(ex[:, start_col:end_col],
                                             lg[:, start_col:end_col],
                                             AF.Exp, scale=0.125)
                        for mi, mb in enumerate(masks):
                            eng = nc.vector
                            eng.tensor_tensor(
                                ex[:, mb:mb + 128], ex[:, mb:mb + 128],
                                tri[:], op=AluOpType.mult)
                        exs[s] = (ex, shift)
                    if g == 0:
                        # previous q-block's division must be emitted before
                        # this block's first AV matmuls (PSUM ring reuse)
                        drain_div()
                    # drain between the logits emission and the previous
                    # group's AV: these pieces have no exp dependency, so they
                    # absorb the PE bubble while AV waits on exp+mask.
                    bucket[0] = min(bucket[0] + 360 * len(slots), 2600)
                    bucket[0] = drain_ns(bucket[0])
                    if prev is not None:
                        emit_avs(*prev)
                    prev = (exs, ta)
                emit_avs(*prev)

                # softmax divide (per-partition: queries on partitions now),
                # transpose back to [dh, q], write into at_all.  Queued as
                # pieces so the next q-block's logits go out first and the
                # DVE/PE/Pool ping-pong here overlaps its groups.
                def mk_div(s, av):
                    def emit_div():
                        recip = dvp.tile([128, 4], F32, tag="recip",
                                         name=f"rc{qb}_{s}")
                        dv = dvp.tile([128, 4 * 64], BF16, tag="dv",
                                      name=f"dv{qb}_{s}")
                        with nc.allow_low_precision(reason="softmax recip"):
                            nc.vector.reciprocal(
                                recip[:],
                                av[:].rearrange("p (j x) -> p j x",
                                                x=65)[:, :, 64])
                        for j in range(4):
                            nc.vector.tensor_scalar(
                                dv[:, j * 64:(j + 1) * 64],
                                av[:, j * 65:j * 65 + 64],
                                recip[:, j:j + 1], None, AluOpType.mult)
                        return dv

                    div_state = {}

                    def piece_a():
                        div_state["dv"] = emit_div()

                    def mk_tp(j):
                        def piece_tp():
                            dv = div_state["dv"]
                            tp = psJ.tile([64, 128], BF16, tag="pj",
                                          name=f"tp{qb}_{s}_{j}")
                            nc.tensor.transpose(
                                tp[:], dv[:, j * 64:(j + 1) * 64], eye[:])
                            nc.vector.tensor_copy(
                                at_all[s * 64:(s + 1) * 64,
                                       qb * QB + j * KT:
                                       qb * QB + (j + 1) * KT],
                                tp[:])
                        return piece_tp

                    return ([(150, piece_a)] +
                            [(120, mk_tp(j)) for j in range(4)])

                for s in slots:
                    div_q.extend(mk_div(s, avs[s]))
                queue_oproj(qb, fast=last)
                if last:
                    drain_all()

            # ---------------- projections (piecewise) -------------------
            def dma_chunk(c, trig=True):
                c0, c1 = c * CH, (c + 1) * CH
                xc_all = xcp.tile([128, 6 * CH], BF16, tag="xc", name=f"xca{c}")
                if c == 0:
                    # split so the first K-proj accumulations start sooner
                    for i0 in (0, 3):
                        nc.sync.dma_start(
                            xc_all[:, i0 * CH:(i0 + 3) * CH].rearrange(
                                "p (i c) -> p i c", c=CH),
                            xt_d[:, c0:c1].rearrange(
                                "(i p) c -> p i c", p=128)[:, i0:i0 + 3])
                else:
                    nc.sync.dma_start(
                        xc_all[:].rearrange("p (i c) -> p i c", c=CH),
                        xt_d[:, c0:c1].rearrange("(i p) c -> p i c", p=128))
                cosf_c = rtp.tile([128, CH], F32, tag="cosc", name=f"cosc{c}")
                sins_c = rtp.tile([128, CH], F32, tag="sinc", name=f"sinc{c}")
                if trig:
                    nc.sync.dma_start(cosf_c[:], cosf_d[:, c0:c1])
                    nc.sync.dma_start(sins_c[:], sins_d[:, c0:c1])
                return xc_all, cosf_c, sins_c

            def proj_pieces(c, dmares):
                c0, c1 = c * CH, (c + 1) * CH
                xc_all, cosf_c, sins_c = dmares
                xc = [xc_all[:, i * CH:(i + 1) * CH] for i in range(6)]
                st = {}

                def rope(ps, dst):
                    tsw = rtp.tile([128, CH], F32, tag="tsw")
                    nc.vector.stream_shuffle(tsw[:], ps[:], ROT16)
                    m1 = rtp.tile([128, CH], F32, tag="m1")
                    nc.vector.tensor_tensor(m1[:], ps[:], cosf_c[:],
                                            op=AluOpType.mult)
                    m2 = rtp.tile([128, CH], F32, tag="m2")
                    nc.gpsimd.tensor_tensor(m2[:], tsw[:], sins_c[:],
                                            op=AluOpType.mult)
                    nc.vector.tensor_tensor(dst[:, c0:c1], m1[:], m2[:],
                                            op=AluOpType.add)

                def mk_qk(key, w_t, i0):
                    def piece():
                        if i0 == 0:
                            st[key] = psJ.tile([128, CH], F32, tag="pj",
                                               name=f"p{key}{c}")
                        ps = st[key]
                        for i in (i0, i0 + 1, i0 + 2):
                            nc.tensor.matmul(ps[:], w_t[i], xc[i],
                                             start=(i == 0), stop=(i == 5))
                        if i0 == 3 and key == "k":
                            rope(st["k"], kt_rot)
                    return piece

                def piece_ropeq():
                    rope(st["q"], qt_rot)

                def mkv(j):
                    def piece_v():
                        # direct V: x-tile stationary -> [token, dh] layout
                        t = 4 * c + j
                        psv = psJ.tile([128, 128], F32, tag="pj",
                                       name=f"pv{c}_{j}")
                        for i in range(6):
                            nc.tensor.matmul(
                                psv[:],
                                xc_all[:, i * CH + j * KT:
                                       i * CH + (j + 1) * KT],
                                wv_t[i], start=(i == 0), stop=(i == 5))
                        vp_dst = vp_all[:, t * VPW:t * VPW + VPW].rearrange(
                            "p (b x) -> p b x", x=65)[:, :, 0:64]
                        vp_src = psv[:].rearrange("p (b x) -> p b x", x=64)
                        if c <= 3:
                            nc.scalar.copy(vp_dst, vp_src)
                        else:
                            nc.vector.tensor_copy(vp_dst, vp_src)
                    return piece_v

                return ([(650, mk_qk("k", wk_t, 0), c),
                         (650, mk_qk("k", wk_t, 3), c),
                         (650, mk_qk("q", wq_t, 0), c),
                         (650, mk_qk("q", wq_t, 3), c),
                         (130, piece_ropeq, c)] +
                        [(380, mkv(j), c) for j in range(4)])

            # ---------------- interleaved main loop ---------------------
            # Process q-blocks so the final one is single-slot (smaller
            # Act-engine tail); projections stream in chunk order through the
            # pending queue, interleaved between attention groups.
            order = list(range(8)) if 7 not in bset else [0, 1, 2, 3, 4, 5, 7, 6]
            dma_res = {0: dma_chunk(0, trig=False)}
            # serial DMA device: K/Q weights before chunk-0 trig tables so the
            # first projections start sooner; wv right after (AV(0) needs it)
            for wall, wd in ((wk_all, wkt_d), (wq_all, wqt_d)):
                nc.sync.dma_start(
                    wall[:].rearrange("p (i c) -> p i c", c=128),
                    wd[:].rearrange("(i p) c -> p i c", p=128))
            nc.sync.dma_start(dma_res[0][1][:], cosf_d[:, 0:CH])
            nc.sync.dma_start(dma_res[0][2][:], sins_d[:, 0:CH])
            nc.sync.dma_start(
                wv_all[:].rearrange("p (i c) -> p i c", c=128),
                wvt_d[:].rearrange("(i p) c -> p i c", p=128))
            dma_res[1] = dma_chunk(1)
            dma_weights_back()
            p0 = proj_pieces(0, dma_res.pop(0))
            for ent in p0[:4]:  # K/Q projections + rope K
                ent[1]()
            proj_q.extend(p0[4:])  # rope Q + V tiles drain inside attn(0)
            chunks_queued = 0
            # queue chunks one window ahead of need where slack allows, so a
            # late q-block never has to absorb two chunks at once
            targets = {0: 1, 1: 2, 2: 4, 3: 5, 4: 6, 5: 7}
            for i, qb in enumerate(order):
                nxt = order[i + 1] if i + 1 < len(order) else -1
                nxt = max(nxt, targets.get(i, 7 if i < len(order) - 1 else -1))
                # queue pieces for every chunk the NEXT attention block needs;
                # they drain between this block's groups (forced at its end)
                while chunks_queued < nxt:
                    chunks_queued += 1
                    if chunks_queued + 1 <= 7 and chunks_queued + 1 not in dma_res:
                        dma_res[chunks_queued + 1] = dma_chunk(chunks_queued + 1)
                    proj_q.extend(
                        proj_pieces(chunks_queued, dma_res.pop(chunks_queued)))
                # leftovers whose chunk the upcoming block itself needs
                drain_proj_upto(qb)
                attention_qb(qb, sched[qb][1], last=(i == len(order) - 1))

            if DEBUG:
                dbg_ktf = cp.tile([128, S], F32, tag="dktf")
                dbg_qtf = cp.tile([128, S], F32, tag="dqtf")
                dbg_vpf = cp.tile([128, NKT * VPW], F32, tag="dvpf")
                dbg_atf = cp.tile([128, S], F32, tag="datf")
                nc.vector.tensor_copy(dbg_ktf[:], kt_rot[:])
                nc.vector.tensor_copy(dbg_qtf[:], qt_rot[:])
                nc.vector.tensor_copy(dbg_vpf[:], vp_all[:])
                nc.vector.tensor_copy(dbg_atf[:], at_all[:])
                nc.sync.dma_start(dbg_kt[:], dbg_ktf[:])
                nc.sync.dma_start(dbg_qt[:], dbg_qtf[:])
                nc.sync.dma_start(dbg_vp[:], dbg_vpf[:])
                nc.sync.dma_start(dbg_at[:], dbg_atf[:])

    nc.compile()
    return nc


def _get_program(bset):
    key = tuple(bset)
    if key not in _PROGRAMS:
        _PROGRAMS[key] = _build_program(key)
    return _PROGRAMS[key]


def _prep_core_inputs(core, x2d_T, token_positions, Wq, Wk, Wv, Wo):
    import ml_dtypes
    bf16 = ml_dtypes.bfloat16
    hA, hB = CORE_HEADS[core]
    pos = token_positions.astype(np.float64)
    inv_freq = 1.0 / (THETA ** (np.arange(0, DH, 2, dtype=np.float64) / DH))  # [32]
    ang = pos[:, None] * inv_freq[None, :]          # [S, 32]
    cosv, sinv = np.cos(ang), np.sin(ang)           # [S, 32]

    cosf = np.empty((128, S), np.float32)
    sins = np.empty((128, S), np.float32)
    for r in range(64):
        q, i = r // 32, r % 32
        f = 16 * q + (i % 16)
        cosf[r] = cosf[r + 64] = cosv[:, f].astype(np.float32)
        sgn = -1.0 if i < 16 else 1.0
        sins[r] = sins[r + 64] = (sgn * sinv[:, f]).astype(np.float32)

    rows = np.concatenate([hA * DH + PERM64, hB * DH + PERM64])
    wqt = np.ascontiguousarray(Wq[rows].T).astype(bf16)   # [768,128]
    wkt = np.ascontiguousarray(Wk[rows].T).astype(bf16)
    vrows = np.concatenate([np.arange(hA * DH, (hA + 1) * DH),
                            np.arange(hB * DH, (hB + 1) * DH)])
    wvt = np.ascontiguousarray(Wv[vrows].T).astype(bf16)  # [768,128]
    wot = np.ascontiguousarray(Wo[:, vrows].T).astype(bf16)  # [128,768]

    tri = np.where(np.arange(128)[None, :] >= np.arange(128)[:, None],
                   1.0, 0.0).astype(bf16)  # [k', q'] 0/1 mask
    return {
        "xt": x2d_T,
        "wqt": wqt, "wkt": wkt, "wvt": wvt, "wot": wot,
        "cosf": cosf, "sins": sins,
        "tri": tri,
        "eye": np.eye(128, dtype=bf16),
    }


def _dispatch_group(nc, in_maps, devices):
    """Async-dispatch one program on a device subset; returns (arrs, names, avals, n)."""
    import jax
    from jax.sharding import Mesh, PartitionSpec
    from concourse import bass2jax, mybir

    bass2jax.install_neuronx_cc_hook()
    n = len(in_maps)
    partition_name = (nc.partition_id_tensor.name
                      if nc.partition_id_tensor else None)
    in_names, out_names, out_avals, zero_outs = [], [], [], []
    for alloc in nc.m.functions[0].allocations:
        if not isinstance(alloc, mybir.MemoryLocationSet):
            continue
        name = alloc.memorylocations[0].name
        if alloc.kind == "ExternalInput":
            if name != partition_name:
                in_names.append(name)
        elif alloc.kind == "ExternalOutput":
            shape = tuple(alloc.tensor_shape)
            dtype = mybir.dt.np(alloc.dtype)
            out_names.append(name)
            out_avals.append(jax.core.ShapedArray(shape, dtype))
            zero_outs.append(np.zeros(shape, dtype))
    n_params = len(in_names)
    all_names = in_names + out_names
    if partition_name is not None:
        all_names = all_names + [partition_name]
    donate = tuple(range(n_params, n_params + len(out_names)))

    def _body(*args):
        operands = list(args)
        if partition_name is not None:
            operands.append(bass2jax.partition_id_tensor())
        outs = bass2jax._bass_exec_p.bind(
            *operands, out_avals=tuple(out_avals), in_names=tuple(all_names),
            out_names=tuple(out_names), lowering_input_output_aliases=(),
            sim_require_finite=True, sim_require_nnan=True, nc=nc)
        return tuple(outs)

    try:
        from jax.experimental.shard_map import shard_map
    except ImportError:
        from jax.shard_map import shard_map  # newer jax

    mesh = Mesh(np.asarray(devices), ("core",))
    in_specs = (PartitionSpec("core"),) * (n_params + len(out_names))
    out_specs = (PartitionSpec("core"),) * len(out_names)
    sharded = jax.jit(
        shard_map(_body, mesh=mesh, in_specs=in_specs, out_specs=out_specs,
                  check_rep=False),
        donate_argnums=donate, keep_unused=True)
    per_core = [[np.asarray(m[nm]) for nm in in_names] for m in in_maps]
    concat_in = [np.concatenate([per_core[c][i] for c in range(n)], axis=0)
                 for i in range(n_params)]
    concat_zeros = [np.zeros((n * z.shape[0], *z.shape[1:]), z.dtype)
                    for z in zero_outs]
    out_arrs = sharded(*concat_in, *concat_zeros)
    return out_arrs, out_names, out_avals, n


def kernel(x, token_positions, Wq, Wk, Wv, Wo):
    import jax
    import ml_dtypes

    x = np.asarray(x)
    token_positions = np.asarray(token_positions)
    Wq, Wk, Wv, Wo = (np.asarray(a, np.float32) for a in (Wq, Wk, Wv, Wo))
    B = x.shape[0]
    assert x.shape == (B, S, D) and B == 1

    x2d_T = np.ascontiguousarray(x[0].T.astype(np.float32)).astype(
        ml_dtypes.bfloat16)  # [768, 4096]

    in_maps = [_prep_core_inputs(c, x2d_T, token_positions, Wq, Wk, Wv, Wo)
               for c in range(8)]

    nc_even = _get_program(BSET_EVEN)
    nc_odd = _get_program(BSET_ODD)

    devs = jax.devices()
    # even program on devices 0-3 <- logical cores 0,2,4,6
    # odd  program on devices 4-7 <- logical cores 1,3,5,7
    g1_maps = [in_maps[c] for c in (0, 2, 4, 6)]
    g2_maps = [in_maps[c] for c in (1, 3, 5, 7)]

    arrs1, names1, avals1, n1 = _dispatch_group(nc_even, g1_maps, devs[0:4])
    arrs2, names2, avals2, n2 = _dispatch_group(nc_odd, g2_maps, devs[4:8])

    def collect(arrs, names, avals, n):
        res = []
        for c in range(n):
            res.append({
                nm: np.asarray(arrs[i]).reshape(n, *avals[i].shape)[c]
                for i, nm in enumerate(names)})
        return res

    res1 = collect(arrs1, names1, avals1, n1)
    res2 = collect(arrs2, names2, avals2, n2)

    acc = np.zeros((D, S), np.float32)
    for r in res1 + res2:
        acc += r["opart"].astype(np.float32)
    out = np.ascontiguousarray(acc.T).reshape(1, S, D)
    return out
